# revision 24
# baseline (speedup 1.0000x reference)
"""Trainium2 Bass kernel for E[b,k,d] = sum_n A[b,n,k] * R[b,n,k,d].

Full shapes: A (16, 8192, 32) f32, R (16, 8192, 32, 64) f32 -> E (16, 32, 64) f32.
Sharding: batch B=16 split across 8 cores (2 batches per core), no collectives.

Strategy (memory-bound problem): quantize R (the 1 GiB stream) down to fp8
e3m4 and keep the DMA pipeline at the measured ~425 GB/s SWDGE rate.

  - R is cast on host to R_DTYPE (fp8 e3m4 1B or bf16 2B per element) and
    transposed to [b, p, c, k*d]: per partition the chunk stream is
    contiguous in DRAM, which the SWDGE packet coalescer turns into 8-16 KiB
    packets (4 KiB packets cost ~8% of engine throughput).
    Error (measured vs f32 reference, deterministic inputs, metric
    max|err|/max|expected|): e3m4 1.441e-2, bf16 2.2e-3 vs the 2e-2 gate.
    fp8 products are exact on the PE (FP22 internal), so the numpy
    simulation of this error transfers bit-comparably to hardware.
  - A stays bf16 (1.5% of traffic), padded per-row by 64 B: a fully
    contiguous [P, C*K] DRAM region (partition stride == run length) gets
    merged into ONE 1-D descriptor that a single DMA engine drains serially
    at ~26 GB/s; the pad keeps the AP 2-D so all 16 engines share it.
  - Per (b, chunk, k-pair j): lhsT = [R_k0 | R_k1] ([128n x 128] fp8,
    stationary, hits Fast Weight Load ~27ns), rhs = [A_k0, A_k1]
    ([128n x 2] bf16; mixed-dtype matmul is legal, both upcast to FP22) ->
    acc[:, 2j:2j+2] accumulates over all n-chunks:
      col 2j   rows 0:64   = sum_n A_k0 * R_k0[d]   (useful)
      col 2j+1 rows 64:128 = sum_n A_k1 * R_k1[d]   (useful)
    (the other half of each column is a discarded cross term).
  - Extraction per b: two strided DVE copies pull the useful halves out of
    PSUM into e2[d, k], two 32x32 DVE transposes -> o[k, d], one 8 KiB store.
  - DMA: everything bulk rides the gpsimd SWDGE queue; alone, with ~1 MiB
    sub-DMAs queued deep, it sustains 410-428 GB/s, while any concurrent
    HWDGE (sync/scalar) traffic drags the aggregate down to ~320-360 (also
    true at the tail -- measured, repeatedly).  Sub-DMAs write disjoint
    slices of the group tile, so matmuls wait per-MiB, not per-4MiB.
    E stores ride sync (8 KiB, negligible).  The SWDGE ring balances
    dma_starts across the 16 engines by queued bytes, so sub sizing is
    uncritical; the ~280 KiB of PE instruction paging always rides one
    engine and makes it the straggler -- unavoidable from kernel side
    (tested: HWDGE tails, equal-size byte-folded streams, multi-queue
    spreads all regress).
"""

import numpy as np

_NC_CACHE = {}

R_DTYPE = "f8e3"  # "bf16" or "f8e3"
_CPQ = {"bf16": 8, "f8e3": 16}  # n-chunks per group tile (~4 MiB)


def _np_rdtype():
    import ml_dtypes

    return {"bf16": ml_dtypes.bfloat16, "f8e3": ml_dtypes.float8_e3m4}[R_DTYPE]


def _pack(A, R):
    """R -> RP[b, p, c*K*D] (R_DTYPE); A -> AP[b, p, C*K + 32 pad] bf16."""
    from concurrent.futures import ThreadPoolExecutor

    import ml_dtypes

    bf16 = ml_dtypes.bfloat16
    rdt = _np_rdtype()
    B, N, K = A.shape
    D = R.shape[-1]
    P = 128
    C = N // P

    RP = np.empty((B, P, C * K * D), dtype=rdt)
    AP = np.zeros((B, P, C * K + 32), dtype=bf16)

    def pack_batch(b):
        RP[b] = (
            R[b].reshape(C, P, K * D).astype(rdt).transpose(1, 0, 2).reshape(P, -1)
        )
        AP[b, :, : C * K] = np.ascontiguousarray(
            A[b].reshape(C, P, K).transpose(1, 0, 2)
        ).reshape(P, C * K)

    with ThreadPoolExecutor(max_workers=16) as ex:
        list(ex.map(pack_batch, range(B)))
    return RP, AP


def _build_nc(Bs, N, K, D, hw_fixups=True):
    import concourse.bass as bass
    import concourse.mybir as mybir
    import concourse.tile as tile

    P = 128
    C = N // P  # n-chunks per batch
    Q = _CPQ[R_DTYPE]  # chunks per group tile
    G = C // Q  # groups per batch
    KD = K * D
    J = K // 2  # k-pairs
    rdt = {"bf16": mybir.dt.bfloat16, "f8e3": mybir.dt.float8e3}[R_DTYPE]

    nc = bass.Bass()
    RP_d = nc.declare_dram_parameter("RP", [Bs, P, C * KD], rdt, isOutput=False)
    AP_d = nc.declare_dram_parameter(
        "AP", [Bs, P, C * K + 32], mybir.dt.bfloat16, isOutput=False
    )
    E_d = nc.declare_dram_parameter("E", [Bs, K, D], mybir.dt.float32, isOutput=True)

    def rp_span(b, c0, c1):
        # chunks c0:c1 as [P, (c1-c0)*KD]: contiguous per partition.
        return RP_d[b, :, c0 * KD : c1 * KD]

    with tile.TileContext(nc) as tc:
        with (
            tc.tile_pool(name="rpool", bufs=4) as rpool,
            tc.tile_pool(name="apool", bufs=2) as apool,
            tc.tile_pool(name="opool", bufs=2) as opool,
            tc.tile_pool(name="psum", bufs=2, space="PSUM") as psum_pool,
        ):
            ats = []
            for b in range(Bs):
                at = apool.tile([P, C * K], mybir.dt.bfloat16, tag="at")
                ats.append(at)
            nc.gpsimd.dma_start(out=ats[0][:], in_=AP_d[0, :, : C * K])

            for b in range(Bs):
                at = ats[b]
                acc = psum_pool.tile([P, K], mybir.dt.float32, tag="acc")
                for g in range(G):
                    rt = rpool.tile([P, Q * KD], rdt, tag="rt")
                    for q0 in range(0, Q, 4):
                        nc.gpsimd.dma_start(
                            out=rt[:, q0 * KD : (q0 + 4) * KD],
                            in_=rp_span(b, g * Q + q0, g * Q + q0 + 4),
                        )
                    if b == 0 and g == 0 and Bs > 1:
                        # A1 right after the first group: early, tiny blip.
                        nc.gpsimd.dma_start(out=ats[1][:], in_=AP_d[1, :, : C * K])
                    for q in range(Q):
                        c = g * Q + q
                        for j in range(J):
                            # One accumulation group per acc tile (start
                            # zeroes the whole PSUM zero-region).
                            nc.tensor.matmul(
                                out=acc[:, 2 * j : 2 * j + 2],
                                lhsT=rt[
                                    :, q * KD + j * 2 * D : q * KD + (j + 1) * 2 * D
                                ],
                                rhs=at[:, c * K + 2 * j : c * K + 2 * j + 2],
                                start=(g == 0 and q == 0 and j == 0),
                                stop=(g == G - 1 and q == Q - 1 and j == J - 1),
                            )
                # Pull the useful halves out of PSUM:
                #   E[2j, d] = acc[d, 2j];  E[2j+1, d] = acc[64+d, 2j+1]
                e2 = opool.tile([D, K], mybir.dt.float32, tag="e2")
                nc.vector.tensor_copy(out=e2[:, 0:K:2], in_=acc[0:D, 0:K:2])
                nc.vector.tensor_copy(out=e2[:, 1:K:2], in_=acc[D : 2 * D, 1:K:2])
                o = opool.tile([K, D], mybir.dt.float32, tag="o")
                for blk in range(D // 32):
                    nc.vector.transpose(
                        out=o[:, blk * 32 : (blk + 1) * 32],
                        in_=e2[blk * 32 : (blk + 1) * 32, :],
                    )
                nc.gpsimd.dma_start(out=E_d[b], in_=o[:])

    if hw_fixups:
        # CoreSim can't digest post-scheduling instruction insertion, so the
        # walrus-only wait splitting is skipped for simulator builds.
        _fix_multiwait_insts(nc, mybir)
    return nc


def _fix_multiwait_insts(nc, mybir):
    """Walrus's 64-byte instruction structs in this lowering path accept only
    ONE sync wait per instruction.

    1. Slot-reusing gpsimd DMAs carry (readers-done, prior-slot-DMA-done)
       wait pairs.  All plain gpsimd dma_starts share SWDGE ring 0 (FIFO per
       SDMA engine), so the prior-DMA (DMASW*) wait is implied by ring order
       and is dropped when another wait remains.
    2. Any instruction still carrying N>1 waits (e.g. the framework's kernel
       tail Drain) is split: N-1 single-wait NoOps are inserted before it on
       the same engine queue, which is semantically identical since each
       engine executes its queue in order."""
    for blk in nc.m.functions[0].blocks:
        new_insts = []
        for inst in blk.instructions:
            si = inst.sync_info
            if si is None or len(si.on_wait) <= 1:
                new_insts.append(inst)
                continue
            waits = list(si.on_wait)
            if (
                type(inst).__name__ == "InstDMACopy"
                and str(inst.engine).split(".")[-1] == "Pool"
            ):
                keep = [w for w in waits if not w.ant_name.startswith("DMASW")]
                if len(keep) == 1:
                    inst.sync_info = mybir.SyncInfo(
                        on_wait=keep, on_update=list(si.on_update)
                    )
                    new_insts.append(inst)
                    continue
                waits = keep or waits
            for w in waits[:-1]:
                new_insts.append(
                    mybir.InstNoOp(
                        name=nc.get_next_instruction_name(),
                        engine=inst.engine,
                        bass_nofuse=True,
                        sync_info=mybir.SyncInfo(on_wait=[w], on_update=[]),
                    )
                )
            inst.sync_info = mybir.SyncInfo(
                on_wait=[waits[-1]], on_update=list(si.on_update)
            )
            new_insts.append(inst)
        blk.instructions[:] = new_insts


def _get_nc(Bs, N, K, D):
    key = (Bs, N, K, D, R_DTYPE)
    if key not in _NC_CACHE:
        _NC_CACHE[key] = _build_nc(Bs, N, K, D)
    return _NC_CACHE[key]


def kernel(A, R, **run_kwargs):
    from concourse.bass_utils import run_bass_kernel_spmd

    A = np.asarray(A, dtype=np.float32)
    R = np.asarray(R, dtype=np.float32)
    B, N, K = A.shape
    D = R.shape[-1]
    n_cores = 8
    Bs = B // n_cores

    nc = _get_nc(Bs, N, K, D)
    RP, AP = _pack(A, R)
    in_maps = [
        {"RP": RP[i * Bs : (i + 1) * Bs], "AP": AP[i * Bs : (i + 1) * Bs]}
        for i in range(n_cores)
    ]
    res = run_bass_kernel_spmd(nc, in_maps, list(range(n_cores)), **run_kwargs)
    out = np.concatenate([res.results[i]["E"] for i in range(n_cores)], axis=0)
    if run_kwargs:
        return out, res
    return out


# revision 25
# speedup vs baseline: 1.0428x; 1.0428x over previous
"""Trainium2 Bass kernel for E[b,k,d] = sum_n A[b,n,k] * R[b,n,k,d].

Full shapes: A (16, 8192, 32) f32, R (16, 8192, 32, 64) f32 -> E (16, 32, 64) f32.
Sharding: batch B=16 split across 8 cores (2 batches per core), no collectives.

Strategy (memory-bound problem): quantize R (the 1 GiB stream) down to fp8
e3m4 and keep the DMA pipeline at the measured ~425 GB/s SWDGE rate.

  - R is cast on host to R_DTYPE (fp8 e3m4 1B or bf16 2B per element) and
    transposed to [b, p, c, k*d]: per partition the chunk stream is
    contiguous in DRAM, which the SWDGE packet coalescer turns into 8-16 KiB
    packets (4 KiB packets cost ~8% of engine throughput).
    Error (measured vs f32 reference, deterministic inputs, metric
    max|err|/max|expected|): e3m4 1.441e-2, bf16 2.2e-3 vs the 2e-2 gate.
    fp8 products are exact on the PE (FP22 internal), so the numpy
    simulation of this error transfers bit-comparably to hardware.
  - A stays bf16 (1.5% of traffic), padded per-row by 64 B: a fully
    contiguous [P, C*K] DRAM region (partition stride == run length) gets
    merged into ONE 1-D descriptor that a single DMA engine drains serially
    at ~26 GB/s; the pad keeps the AP 2-D so all 16 engines share it.
  - Per (b, chunk, k-pair j): lhsT = [R_k0 | R_k1] ([128n x 128] fp8,
    stationary, hits Fast Weight Load ~27ns), rhs = [A_k0, A_k1]
    ([128n x 2] bf16; mixed-dtype matmul is legal, both upcast to FP22) ->
    acc[:, 2j:2j+2] accumulates over all n-chunks:
      col 2j   rows 0:64   = sum_n A_k0 * R_k0[d]   (useful)
      col 2j+1 rows 64:128 = sum_n A_k1 * R_k1[d]   (useful)
    (the other half of each column is a discarded cross term).
  - Extraction per b: two strided DVE copies pull the useful halves out of
    PSUM into e2[d, k], two 32x32 DVE transposes -> o[k, d], one 8 KiB store.
  - DMA: everything bulk rides the gpsimd SWDGE queue; alone, with ~1 MiB
    sub-DMAs queued deep, it sustains 410-428 GB/s, while any concurrent
    HWDGE (sync/scalar) traffic drags the aggregate down to ~320-360 (also
    true at the tail -- measured, repeatedly).  Sub-DMAs write disjoint
    slices of the group tile, so matmuls wait per-MiB, not per-4MiB.
    E stores ride sync (8 KiB, negligible).  The SWDGE ring balances
    dma_starts across the 16 engines by queued bytes, so sub sizing is
    uncritical; the ~280 KiB of PE instruction paging always rides one
    engine and makes it the straggler -- unavoidable from kernel side
    (tested: HWDGE tails, equal-size byte-folded streams, multi-queue
    spreads all regress).
"""

import numpy as np

_NC_CACHE = {}

R_DTYPE = "f8e3"  # "bf16" or "f8e3"
_CPQ = {"bf16": 8, "f8e3": 16}  # n-chunks per group tile (~4 MiB)


def _np_rdtype():
    import ml_dtypes

    return {"bf16": ml_dtypes.bfloat16, "f8e3": ml_dtypes.float8_e3m4}[R_DTYPE]


def _pack(A, R):
    """R -> RP[b, p, c*K*D] (R_DTYPE); A -> AP[b, p, C*K + 32 pad] bf16."""
    from concurrent.futures import ThreadPoolExecutor

    import ml_dtypes

    bf16 = ml_dtypes.bfloat16
    rdt = _np_rdtype()
    B, N, K = A.shape
    D = R.shape[-1]
    P = 128
    C = N // P

    RP = np.empty((B, P, C * K * D), dtype=rdt)
    AP = np.zeros((B, P, C * K + 32), dtype=bf16)

    def pack_batch(b):
        RP[b] = (
            R[b].reshape(C, P, K * D).astype(rdt).transpose(1, 0, 2).reshape(P, -1)
        )
        AP[b, :, : C * K] = np.ascontiguousarray(
            A[b].reshape(C, P, K).transpose(1, 0, 2)
        ).reshape(P, C * K)

    with ThreadPoolExecutor(max_workers=16) as ex:
        list(ex.map(pack_batch, range(B)))
    return RP, AP


def _build_nc(Bs, N, K, D, hw_fixups=True):
    import concourse.bass as bass
    import concourse.mybir as mybir
    import concourse.tile as tile

    P = 128
    C = N // P  # n-chunks per batch
    Q = _CPQ[R_DTYPE]  # chunks per group tile
    G = C // Q  # groups per batch
    KD = K * D
    J = K // 2  # k-pairs
    rdt = {"bf16": mybir.dt.bfloat16, "f8e3": mybir.dt.float8e3}[R_DTYPE]

    nc = bass.Bass()
    RP_d = nc.declare_dram_parameter("RP", [Bs, P, C * KD], rdt, isOutput=False)
    AP_d = nc.declare_dram_parameter(
        "AP", [Bs, P, C * K + 32], mybir.dt.bfloat16, isOutput=False
    )
    E_d = nc.declare_dram_parameter("E", [Bs, K, D], mybir.dt.float32, isOutput=True)

    def rp_span(b, c0, c1):
        # chunks c0:c1 as [P, (c1-c0)*KD]: contiguous per partition.
        return RP_d[b, :, c0 * KD : c1 * KD]

    with tile.TileContext(nc) as tc:
        with (
            tc.tile_pool(name="rpool", bufs=4) as rpool,
            tc.tile_pool(name="apool", bufs=2) as apool,
            tc.tile_pool(name="opool", bufs=2) as opool,
            tc.tile_pool(name="psum", bufs=2, space="PSUM") as psum_pool,
        ):
            ats = []
            for b in range(Bs):
                at = apool.tile([P, C * K], mybir.dt.bfloat16, tag="at")
                ats.append(at)
            nc.gpsimd.dma_start(out=ats[0][:], in_=AP_d[0, :, : C * K])

            for b in range(Bs):
                at = ats[b]
                acc = psum_pool.tile([P, K], mybir.dt.float32, tag="acc")
                for g in range(G):
                    rt = rpool.tile([P, Q * KD], rdt, tag="rt")
                    for q0 in range(0, Q, 4):
                        nc.gpsimd.dma_start(
                            out=rt[:, q0 * KD : (q0 + 4) * KD],
                            in_=rp_span(b, g * Q + q0, g * Q + q0 + 4),
                        )
                    if b == 0 and g == 0 and Bs > 1:
                        # A1 right after the first group: early, tiny blip.
                        nc.gpsimd.dma_start(out=ats[1][:], in_=AP_d[1, :, : C * K])
                    for q in range(Q):
                        c = g * Q + q
                        for j in range(J):
                            # One accumulation group per acc tile (start
                            # zeroes the whole PSUM zero-region).
                            nc.tensor.matmul(
                                out=acc[:, 2 * j : 2 * j + 2],
                                lhsT=rt[
                                    :, q * KD + j * 2 * D : q * KD + (j + 1) * 2 * D
                                ],
                                rhs=at[:, c * K + 2 * j : c * K + 2 * j + 2],
                                start=(g == 0 and q == 0 and j == 0),
                                stop=(g == G - 1 and q == Q - 1 and j == J - 1),
                            )
                # Pull the useful halves out of PSUM:
                #   E[2j, d] = acc[d, 2j];  E[2j+1, d] = acc[64+d, 2j+1]
                e2 = opool.tile([D, K], mybir.dt.float32, tag="e2")
                nc.vector.tensor_copy(out=e2[:, 0:K:2], in_=acc[0:D, 0:K:2])
                nc.vector.tensor_copy(out=e2[:, 1:K:2], in_=acc[D : 2 * D, 1:K:2])
                o = opool.tile([K, D], mybir.dt.float32, tag="o")
                for blk in range(D // 32):
                    nc.vector.transpose(
                        out=o[:, blk * 32 : (blk + 1) * 32],
                        in_=e2[blk * 32 : (blk + 1) * 32, :],
                    )
                nc.sync.dma_start(out=E_d[b], in_=o[:])

    if hw_fixups:
        # CoreSim can't digest post-scheduling instruction insertion, so the
        # walrus-only wait splitting is skipped for simulator builds.
        _fix_multiwait_insts(nc, mybir)
    return nc


def _fix_multiwait_insts(nc, mybir):
    """Walrus's 64-byte instruction structs in this lowering path accept only
    ONE sync wait per instruction.

    1. Slot-reusing gpsimd DMAs carry (readers-done, prior-slot-DMA-done)
       wait pairs.  All plain gpsimd dma_starts share SWDGE ring 0 (FIFO per
       SDMA engine), so the prior-DMA (DMASW*) wait is implied by ring order
       and is dropped when another wait remains.
    2. Any instruction still carrying N>1 waits (e.g. the framework's kernel
       tail Drain) is split: N-1 single-wait NoOps are inserted before it on
       the same engine queue, which is semantically identical since each
       engine executes its queue in order."""
    for blk in nc.m.functions[0].blocks:
        new_insts = []
        for inst in blk.instructions:
            si = inst.sync_info
            if si is None or len(si.on_wait) <= 1:
                new_insts.append(inst)
                continue
            waits = list(si.on_wait)
            if (
                type(inst).__name__ == "InstDMACopy"
                and str(inst.engine).split(".")[-1] == "Pool"
            ):
                keep = [w for w in waits if not w.ant_name.startswith("DMASW")]
                if len(keep) == 1:
                    inst.sync_info = mybir.SyncInfo(
                        on_wait=keep, on_update=list(si.on_update)
                    )
                    new_insts.append(inst)
                    continue
                waits = keep or waits
            for w in waits[:-1]:
                new_insts.append(
                    mybir.InstNoOp(
                        name=nc.get_next_instruction_name(),
                        engine=inst.engine,
                        bass_nofuse=True,
                        sync_info=mybir.SyncInfo(on_wait=[w], on_update=[]),
                    )
                )
            inst.sync_info = mybir.SyncInfo(
                on_wait=[waits[-1]], on_update=list(si.on_update)
            )
            new_insts.append(inst)
        blk.instructions[:] = new_insts


def _get_nc(Bs, N, K, D):
    key = (Bs, N, K, D, R_DTYPE)
    if key not in _NC_CACHE:
        _NC_CACHE[key] = _build_nc(Bs, N, K, D)
    return _NC_CACHE[key]


def kernel(A, R, **run_kwargs):
    from concourse.bass_utils import run_bass_kernel_spmd

    A = np.asarray(A, dtype=np.float32)
    R = np.asarray(R, dtype=np.float32)
    B, N, K = A.shape
    D = R.shape[-1]
    n_cores = 8
    Bs = B // n_cores

    nc = _get_nc(Bs, N, K, D)
    RP, AP = _pack(A, R)
    in_maps = [
        {"RP": RP[i * Bs : (i + 1) * Bs], "AP": AP[i * Bs : (i + 1) * Bs]}
        for i in range(n_cores)
    ]
    res = run_bass_kernel_spmd(nc, in_maps, list(range(n_cores)), **run_kwargs)
    out = np.concatenate([res.results[i]["E"] for i in range(n_cores)], axis=0)
    if run_kwargs:
        return out, res
    return out
